# revision 14
# baseline (speedup 1.0000x reference)
"""Self-contained Trainium2 Bass kernel for a dense transformer block.

Shapes (hardcoded from the problem spec):
  x: [4, 2048, 1024], 16 heads x 64, FFN 4096, f32.

Sharding over 8 NeuronCores: core c -> (batch b=c//2, head-half hh=c%2).
Each core computes LN1 on the full sequence of its batch, Q/K/V + causal
attention for its 8 heads over the full sequence, a partial output
projection, then a ReduceScatter within the (2b, 2b+1) pair splits tokens
in half for the residual + LN2 + FFN, producing 1024 output tokens per
core. Everything on-device runs in a transposed [feature, token] layout so
no transposes are ever needed; per-token LN/softmax statistics are reduced
with ones-matmuls on the tensor engine and broadcast back with K=1/K=97
matmuls.
"""

import numpy as np
import ml_dtypes

D = 1024
S = 2048
B = 4
SZ = 64
FF = 4096
P = 128
HC = 8            # heads per core
NPAIR = 4         # head pairs per core
NGRP = 2          # groups of 4 heads
DT = D // P       # 8 d-tiles
FT = FF // P      # 32 ff tiles
TBLK = S // P     # 16 token blocks
QW = 512          # q chunk width
QC = S // QW      # 4 q chunks
SH = S // 2       # tokens per core after reduce-scatter
FTOK = 512        # features (heads*SZ) per core

_CACHE = {}


def _build_program(for_sim=False):
    import concourse.bacc as bacc
    import concourse.mybir as mybir
    import concourse.tile as tile

    dt = mybir.dt
    F32R = dt.float32r
    BF16 = dt.bfloat16
    AF = mybir.ActivationFunctionType
    ALU = mybir.AluOpType

    nc = bacc.Bacc("TRN2", debug=False)

    def din(name, shape, dtype=F32R):
        return nc.dram_tensor(name, shape, dtype, kind="ExternalInput")

    XT = din("XT", [D, S])
    XRES = din("XRES", [D, SH])
    WQT = din("WQT", [D, FTOK], BF16)
    WKT = din("WKT", [D, FTOK], BF16)
    WVT = din("WVT", [D, FTOK], BF16)
    F32 = dt.float32
    BQ = din("BQ", [P, NPAIR], F32)
    BK = din("BK", [P, NPAIR], F32)
    BV = din("BV", [1, FTOK], BF16)
    WOT = din("WOT", [FTOK, D])
    BO = din("BO", [P, DT], F32)
    G1 = din("G1", [P, DT], F32)
    BE1 = din("BE1", [P, DT], F32)
    G2 = din("G2", [P, DT], F32)
    BE2 = din("BE2", [P, DT], F32)
    W1T = din("W1T", [D, FF])
    BF1 = din("BF1", [P, FT], F32)
    W2T = din("W2T", [FF, D], BF16)
    BF2 = din("BF2", [P, DT], F32)
    ONESC = din("ONESC", [P, 1])
    ONESCB = din("ONESCB", [P, 1], BF16)
    ONESR = din("ONESR", [1, P])
    ONESRB = din("ONESRB", [1, P], BF16)
    SELS = din("SELS", [97, 2 * P])
    R40 = din("R40", [97, QW])
    MASKS = din("MASKS", [P, 4, QW], BF16)
    OUT = nc.dram_tensor("OUT", [D, SH], F32R, kind="ExternalOutput")

    io = locals()
    io["for_sim"] = for_sim
    with tile.TileContext(nc) as tc:
        _body(nc, tc, mybir, io)
    nc.compile()
    return nc


def _body(nc, tc, mybir, io):
    dt = mybir.dt
    F32R = dt.float32r
    F32 = dt.float32
    BF16 = dt.bfloat16
    AF = mybir.ActivationFunctionType
    ALU = mybir.AluOpType
    act = nc.scalar.activation
    mm = nc.tensor.matmul
    vec = nc.vector

    XT, XRES = io["XT"], io["XRES"]
    WQT, WKT, WVT = io["WQT"], io["WKT"], io["WVT"]
    BQ, BK, BV = io["BQ"], io["BK"], io["BV"]
    WOT, BO = io["WOT"], io["BO"]
    G1, BE1, G2, BE2 = io["G1"], io["BE1"], io["G2"], io["BE2"]
    W1T, BF1, W2T, BF2 = io["W1T"], io["BF1"], io["W2T"], io["BF2"]
    OUT = io["OUT"]

    # ---------------- constants / biases (whole kernel) ----------------
    consts = tc.alloc_tile_pool(name="consts", bufs=1)
    dram = tc.alloc_tile_pool(name="dram", bufs=1, space="DRAM")

    ones_col = consts.tile([P, 1], F32R)
    nc.sync.dma_start(out=ones_col, in_=io["ONESC"].ap())
    ones_col_bf = consts.tile([P, 1], BF16)
    nc.sync.dma_start(out=ones_col_bf, in_=io["ONESCB"].ap())
    ones_row = consts.tile([1, P], F32R)
    nc.sync.dma_start(out=ones_row, in_=io["ONESR"].ap())
    ones_row_bf = consts.tile([1, P], BF16)
    nc.sync.dma_start(out=ones_row_bf, in_=io["ONESRB"].ap())

    # selector matrices to broadcast reciprocal denominators (rows 0/32/64/96
    # of r4) to the 64-row bands of a head pair
    sels = consts.tile([97, 2 * P], F32R)
    nc.sync.dma_start(out=sels, in_=io["SELS"].ap())
    selA = sels[:, 0:P]
    selB = sels[:, P:2 * P]
    r4 = consts.tile([97, QW], F32R)
    nc.sync.dma_start(out=r4, in_=io["R40"].ap())

    # causal masks for the 4 diagonal k-tiles of a q-chunk:
    # keep exp[k, q] where (q - k - 128*i) >= 0
    maskt = consts.tile([P, 4, QW], BF16)
    nc.sync.dma_start(out=maskt, in_=io["MASKS"].ap())
    masks = [maskt[:, i, :] for i in range(4)]

    def cload(name, src, shape, dtype=F32):
        t = consts.tile(shape, dtype, tag=name)
        nc.sync.dma_start(out=t, in_=src.ap())
        return t

    bq_sb = cload("bq", BQ, [P, NPAIR])
    bk_sb = cload("bk", BK, [P, NPAIR])
    bv_sb = cload("bv", BV, [1, FTOK], BF16)
    bo_sb = cload("bo", BO, [P, DT])
    g1_sb = cload("g1", G1, [P, DT])
    be1_sb = cload("be1", BE1, [P, DT])
    g2_sb = cload("g2", G2, [P, DT])
    be2_sb = cload("be2", BE2, [P, DT])
    bf1_sb = cload("bf1", BF1, [P, FT])
    bf2_sb = cload("bf2", BF2, [P, DT])

    p_dram = dram.tile([2, D, SH], F32R)
    r_dram = dram.tile([D, SH], F32R)

    # ---------------- phase B: LN1 + QKV ----------------
    xn1p = tc.alloc_tile_pool(name="xn1p", bufs=1)
    wqkv = tc.alloc_tile_pool(name="wqkv", bufs=1)
    xn1 = xn1p.tile([P, DT, S], BF16)
    wq_sb = wqkv.tile([P, DT, FTOK], BF16, tag="wq")
    wk_sb = wqkv.tile([P, DT, FTOK], BF16, tag="wk")
    wv_sb = wqkv.tile([P, DT, FTOK], BF16, tag="wv")
    nc.sync.dma_start(out=wq_sb, in_=WQT.ap().rearrange("(t p) f -> p t f", p=P))
    nc.sync.dma_start(out=wk_sb, in_=WKT.ap().rearrange("(t p) f -> p t f", p=P))
    nc.sync.dma_start(out=wv_sb, in_=WVT.ap().rearrange("(t p) f -> p t f", p=P))

    # storage that lives QKV -> attention (right side of the heap)
    kqv = tc.alloc_tile_pool(name="kqv", bufs=1, side="right")
    kt_sb = kqv.tile([P, NPAIR, S], BF16, tag="kt")
    qt_sb = kqv.tile([P, NPAIR, S], BF16, tag="qt")
    v_sb = kqv.tile([P, TBLK, FTOK], BF16, tag="v")

    # LN1: stream x^T tiles, per-token stats via ones-matmuls, normalize
    xa = tc.alloc_tile_pool(name="xa", bufs=2 * DT, side="right")
    sqp = tc.alloc_tile_pool(name="sqp", bufs=3, side="right")
    stp = tc.alloc_tile_pool(name="stp", bufs=2, side="right")
    ln_ps = tc.alloc_tile_pool(name="ln_ps", bufs=2, space="PSUM")
    lnb_ps = tc.alloc_tile_pool(name="lnb_ps", bufs=4, space="PSUM")

    def ln_stats_and_apply(get_in_tile, out_tile, g_sb, be_sb, n_chunks):
        """LN over feature dim for transposed [d, token] tiles.

        get_in_tile(d, qc) -> [P, QW] tile of raw x^T; writes normalized
        bf16/f32r out_tile[:, d, qc*QW:...].
        """
        for qc in range(n_chunks):
            xts = [get_in_tile(d, qc) for d in range(DT)]
            ps_s = ln_ps.tile([1, QW], dt.float32, tag="stat")
            ps_q = ln_ps.tile([1, QW], dt.float32, tag="stat")
            for d in range(DT):
                mm(ps_s, ones_col, xts[d], start=(d == 0), stop=(d == DT - 1))
            for d in range(DT):
                sq = sqp.tile([P, QW], F32R, tag="sq")
                vec.tensor_mul(sq, xts[d], xts[d])
                mm(ps_q, ones_col, sq, start=(d == 0), stop=(d == DT - 1))
            mu = stp.tile([1, QW], F32R, tag="mu")
            act(out=mu, in_=ps_s, func=AF.Copy, scale=1.0 / D)
            msq = stp.tile([1, QW], F32R, tag="msq")
            act(out=msq, in_=ps_q, func=AF.Copy, scale=1.0 / D)
            mu2 = stp.tile([1, QW], F32R, tag="mu2")
            vec.tensor_mul(mu2, mu, mu)
            var = stp.tile([1, QW], F32R, tag="var")
            vec.tensor_sub(var, msq, mu2)
            sd = stp.tile([1, QW], F32R, tag="sd")
            # unbiased variance (ddof=1), eps added after sqrt
            act(out=sd, in_=var, func=AF.Sqrt, scale=float(D) / (D - 1))
            vec.tensor_scalar_add(out=sd, in0=sd, scalar1=1e-8)
            rstd = stp.tile([1, QW], F32R, tag="rstd")
            with nc.allow_low_precision(reason="f32r output is f32-width"):
                vec.reciprocal(out=rstd, in_=sd)
            ps_mu = lnb_ps.tile([P, QW], dt.float32, tag="bc")
            mm(ps_mu, ones_row, mu)
            ps_rs = lnb_ps.tile([P, QW], dt.float32, tag="bc")
            mm(ps_rs, ones_row, rstd)
            for d in range(DT):
                t1 = sqp.tile([P, QW], F32R, tag="sq")
                vec.tensor_sub(t1, xts[d], ps_mu)
                vec.tensor_mul(t1, t1, ps_rs)
                vec.tensor_scalar(
                    out=out_tile[:, d, qc * QW:(qc + 1) * QW], in0=t1,
                    scalar1=g_sb[:, d:d + 1], scalar2=be_sb[:, d:d + 1],
                    op0=ALU.mult, op1=ALU.add,
                )

    def ln1_in(d, qc):
        t = xa.tile([P, QW], F32R, tag="xa")
        nc.sync.dma_start(out=t, in_=XT[d * P:(d + 1) * P, qc * QW:(qc + 1) * QW])
        return t

    ln_stats_and_apply(ln1_in, xn1, g1_sb, be1_sb, QC)
    lnb_ps.release()
    ln_ps.release()

    # QKV projections (bf16). K^T/Q^T: [2*SZ, token] pair-packed tiles.
    qkv_ps = tc.alloc_tile_pool(name="qkv_ps", bufs=3, space="PSUM")
    for p in range(NPAIR):
        for qc in range(QC):
            qs = slice(qc * QW, (qc + 1) * QW)
            for dst, w, b in ((kt_sb, wk_sb, bk_sb), (qt_sb, wq_sb, bq_sb)):
                ps = qkv_ps.tile([P, QW], dt.float32, tag="qkv")
                for d in range(DT):
                    mm(ps, w[:, d, p * P:(p + 1) * P], xn1[:, d, qs],
                       start=(d == 0), stop=(d == DT - 1))
                act(out=dst[:, p, qs], in_=ps, func=AF.Identity, bias=b[:, p:p + 1])
    # V in token-major layout [token, head*sz], bias via rank-1 ones x bv
    for tb in range(TBLK):
        for fc in range(2):
            fs = slice(fc * 256, (fc + 1) * 256)
            ps = qkv_ps.tile([P, 256], dt.float32, tag="qkvv")
            for d in range(DT):
                mm(ps, xn1[:, d, tb * P:(tb + 1) * P], wv_sb[:, d, fs],
                   start=(d == 0), stop=False)
            mm(ps, ones_row_bf, bv_sb[:, fs], start=False, stop=True)
            act(out=v_sb[:, tb, fs], in_=ps, func=AF.Copy)

    qkv_ps.release()
    stp.release()
    sqp.release()
    xa.release()
    wqkv.release()
    xn1p.release()

    # ---------------- attention ----------------
    # all in transposed score layout: exp_t[k, q] = exp(K q^T / 8), causal
    # masked; o^T accumulated per pair; denominators via col-tiled
    # ones-matmuls into psum_d rows {0,32,64,96}.
    ot_p = tc.alloc_tile_pool(name="ot_p", bufs=1)
    ot_sb = ot_p.tile([P, NPAIR, S], F32R)
    expp = tc.alloc_tile_pool(name="expp", bufs=3)
    sc_ps = tc.alloc_tile_pool(name="sc_ps", bufs=5, space="PSUM")
    o_ps = tc.alloc_tile_pool(name="o_ps", bufs=2, space="PSUM")
    d_ps = tc.alloc_tile_pool(name="d_ps", bufs=1, space="PSUM")

    for g in range(NGRP):
        for qc in range(QC):
            qs = slice(qc * QW, (qc + 1) * QW)
            ktiles = 4 * (qc + 1)
            ps_o = [o_ps.tile([P, QW], dt.float32, tag="o", name=f"ps_o{_i}") for _i in range(2)]
            ps_d = d_ps.tile([97, QW], dt.float32, tag="d")
            for kt in range(ktiles):
                ks = slice(kt * P, (kt + 1) * P)
                exp_t = expp.tile([P, 4, QW], BF16, tag="exp")
                for lp in range(2):
                    pr = 2 * g + lp
                    for half in range(2):
                        hs = slice(half * SZ, (half + 1) * SZ)
                        ps_s = sc_ps.tile([P, QW], dt.float32, tag="sc")
                        mm(ps_s, kt_sb[hs, pr, ks], qt_sb[hs, pr, qs])
                        act(out=exp_t[:, 2 * lp + half, :], in_=ps_s,
                            func=AF.Exp, scale=0.125)
                if kt >= ktiles - 4:
                    mi = kt - (ktiles - 4)
                    for lh in range(4):
                        vec.tensor_mul(exp_t[:, lh, :], exp_t[:, lh, :], masks[mi])
                first, last = kt == 0, kt == ktiles - 1
                for lp in range(2):
                    pr = 2 * g + lp
                    for half in range(2):
                        mm(ps_o[lp][half * SZ:(half + 1) * SZ, :],
                           v_sb[:, kt, pr * P + half * SZ:pr * P + (half + 1) * SZ],
                           exp_t[:, 2 * lp + half, :],
                           start=first, stop=last, skip_group_check=True)
                for lh in range(4):
                    mm(ps_d[32 * lh:32 * lh + 1, :], ones_col_bf, exp_t[:, lh, :],
                       start=first, stop=last, tile_position=(0, 32 * lh),
                       skip_group_check=True)
            with nc.allow_low_precision(reason="f32r output is f32-width"):
                for lh in range(4):
                    vec.reciprocal(out=r4[32 * lh:32 * lh + 1, :],
                                   in_=ps_d[32 * lh:32 * lh + 1, :])
            for lp, sel in ((0, selA), (1, selB)):
                ps_b = sc_ps.tile([P, QW], dt.float32, tag="sc")
                mm(ps_b, sel, r4)
                ob = expp.tile([P, QW], F32R, tag="onorm")
                act(out=ob, in_=ps_o[lp], func=AF.Copy)
                vec.tensor_mul(ot_sb[:, 2 * g + lp, qs], ob, ps_b)

    d_ps.release()
    o_ps.release()
    sc_ps.release()
    expp.release()
    kqv.release()

    # ---------------- partial Wo -> p_dram, ReduceScatter ----------------
    wop = tc.alloc_tile_pool(name="wop", bufs=1)
    wo_sb = wop.tile([P, NPAIR, D], F32R)
    nc.sync.dma_start(out=wo_sb, in_=WOT.ap().rearrange("(t p) f -> p t f", p=P))
    pst = tc.alloc_tile_pool(name="pst", bufs=3)
    wo_ps = tc.alloc_tile_pool(name="wo_ps", bufs=2, space="PSUM")
    for d in range(DT):
        for qc in range(QC):
            qs = slice(qc * QW, (qc + 1) * QW)
            ps = wo_ps.tile([P, QW], dt.float32, tag="wo")
            for f in range(NPAIR):
                mm(ps, wo_sb[:, f, d * P:(d + 1) * P], ot_sb[:, f, qs],
                   start=(f == 0), stop=(f == NPAIR - 1))
            st = pst.tile([P, QW], F32R, tag="pst")
            act(out=st, in_=ps, func=AF.Copy)
            nc.sync.dma_start(
                out=p_dram[qc // 2, d * P:(d + 1) * P,
                           (qc % 2) * QW:(qc % 2) * QW + QW],
                in_=st)
    wo_ps.release()
    pst.release()
    wop.release()
    ot_p.release()

    if io.get("for_sim"):
        nc.sync.dma_start(out=r_dram, in_=p_dram[0])
    else:
        nc.gpsimd.collective_compute(
            "ReduceScatter", ALU.add,
            replica_groups=[[0, 1], [2, 3], [4, 5], [6, 7]],
            ins=[p_dram.opt()], outs=[r_dram.opt()],
        )

    # ---------------- residual + LN2 ----------------
    x2p = tc.alloc_tile_pool(name="x2p", bufs=1)
    x2 = x2p.tile([P, DT, SH], F32R)
    hp = tc.alloc_tile_pool(name="hp", bufs=1)
    h_sb = hp.tile([P, FT, SH], BF16)
    xn2p = tc.alloc_tile_pool(name="xn2p", bufs=1)
    xn2 = xn2p.tile([P, DT, SH], F32R)

    rsp = tc.alloc_tile_pool(name="rsp", bufs=6, side="right")
    for d in range(DT):
        for qc in range(SH // QW):
            qs = slice(qc * QW, (qc + 1) * QW)
            rt = rsp.tile([P, QW], F32R, tag="rt")
            nc.sync.dma_start(out=rt, in_=r_dram[d * P:(d + 1) * P, qs])
            xr = rsp.tile([P, QW], F32R, tag="xr")
            nc.sync.dma_start(out=xr, in_=XRES[d * P:(d + 1) * P, qs])
            vec.tensor_add(rt, rt, xr)
            act(out=x2[:, d, qs], in_=rt, func=AF.Identity, bias=bo_sb[:, d:d + 1])
    rsp.release()

    sqp2 = tc.alloc_tile_pool(name="sqp2", bufs=3, side="right")
    stp2 = tc.alloc_tile_pool(name="stp2", bufs=2, side="right")
    ln_ps2 = tc.alloc_tile_pool(name="ln_ps2", bufs=2, space="PSUM")
    lnb_ps2 = tc.alloc_tile_pool(name="lnb_ps2", bufs=4, space="PSUM")

    # reuse LN helper with x2 slices as inputs
    sqp_ref, stp_ref = sqp2, stp2

    def ln2_in(d, qc):
        return x2[:, d, qc * QW:(qc + 1) * QW]

    def ln_stats_and_apply2(get_in_tile, out_tile, g_sb, be_sb, n_chunks):
        for qc in range(n_chunks):
            xts = [get_in_tile(d, qc) for d in range(DT)]
            ps_s = ln_ps2.tile([1, QW], dt.float32, tag="stat")
            ps_q = ln_ps2.tile([1, QW], dt.float32, tag="stat")
            for d in range(DT):
                mm(ps_s, ones_col, xts[d], start=(d == 0), stop=(d == DT - 1))
            for d in range(DT):
                sq = sqp_ref.tile([P, QW], F32R, tag="sq")
                vec.tensor_mul(sq, xts[d], xts[d])
                mm(ps_q, ones_col, sq, start=(d == 0), stop=(d == DT - 1))
            mu = stp_ref.tile([1, QW], F32R, tag="mu")
            act(out=mu, in_=ps_s, func=AF.Copy, scale=1.0 / D)
            msq = stp_ref.tile([1, QW], F32R, tag="msq")
            act(out=msq, in_=ps_q, func=AF.Copy, scale=1.0 / D)
            mu2 = stp_ref.tile([1, QW], F32R, tag="mu2")
            vec.tensor_mul(mu2, mu, mu)
            var = stp_ref.tile([1, QW], F32R, tag="var")
            vec.tensor_sub(var, msq, mu2)
            sd = stp_ref.tile([1, QW], F32R, tag="sd")
            act(out=sd, in_=var, func=AF.Sqrt, scale=float(D) / (D - 1))
            vec.tensor_scalar_add(out=sd, in0=sd, scalar1=1e-8)
            rstd = stp_ref.tile([1, QW], F32R, tag="rstd")
            with nc.allow_low_precision(reason="f32r output is f32-width"):
                vec.reciprocal(out=rstd, in_=sd)
            ps_mu = lnb_ps2.tile([P, QW], dt.float32, tag="bc")
            mm(ps_mu, ones_row, mu)
            ps_rs = lnb_ps2.tile([P, QW], dt.float32, tag="bc")
            mm(ps_rs, ones_row, rstd)
            for d in range(DT):
                t1 = sqp_ref.tile([P, QW], F32R, tag="sq")
                vec.tensor_sub(t1, xts[d], ps_mu)
                vec.tensor_mul(t1, t1, ps_rs)
                vec.tensor_scalar(
                    out=out_tile[:, d, qc * QW:(qc + 1) * QW], in0=t1,
                    scalar1=g_sb[:, d:d + 1], scalar2=be_sb[:, d:d + 1],
                    op0=ALU.mult, op1=ALU.add,
                )

    ln_stats_and_apply2(ln2_in, xn2, g2_sb, be2_sb, SH // QW)
    lnb_ps2.release()
    ln_ps2.release()

    # ---------------- FFN ----------------
    w1p = tc.alloc_tile_pool(name="w1p", bufs=3, side="right")
    ffn_ps = tc.alloc_tile_pool(name="ffn_ps", bufs=3, space="PSUM")
    w1_view = W1T.ap().rearrange("(t p) f -> p t f", p=P)
    for fr in range(FT):
        w1t = w1p.tile([P, DT, P], F32R, tag="w1")
        nc.sync.dma_start(out=w1t, in_=w1_view[:, :, fr * P:(fr + 1) * P])
        for qc in range(SH // QW):
            qs = slice(qc * QW, (qc + 1) * QW)
            ps = ffn_ps.tile([P, QW], dt.float32, tag="ffn")
            for d in range(DT):
                mm(ps, w1t[:, d, :], xn2[:, d, qs],
                   start=(d == 0), stop=(d == DT - 1))
            act(out=h_sb[:, fr, qs], in_=ps, func=AF.Gelu,
                bias=bf1_sb[:, fr:fr + 1])
    w1p.release()
    stp2.release()
    sqp2.release()
    xn2p.release()

    w2p = tc.alloc_tile_pool(name="w2p", bufs=2, side="right")
    outp = tc.alloc_tile_pool(name="outp", bufs=3)
    w2_view = W2T.ap().rearrange("(t p) e -> p t e", p=P)
    for d in range(DT):
        w2t = w2p.tile([P, FT, P], BF16, tag="w2")
        nc.sync.dma_start(out=w2t, in_=w2_view[:, :, d * P:(d + 1) * P])
        for qc in range(SH // QW):
            qs = slice(qc * QW, (qc + 1) * QW)
            ps = ffn_ps.tile([P, QW], dt.float32, tag="ffn")
            for f in range(FT):
                mm(ps, w2t[:, f, :], h_sb[:, f, qs],
                   start=(f == 0), stop=(f == FT - 1))
            ot = outp.tile([P, QW], F32R, tag="ot")
            act(out=ot, in_=ps, func=AF.Identity, bias=bf2_sb[:, d:d + 1])
            vec.tensor_add(ot, ot, x2[:, d, qs])
            nc.sync.dma_start(out=OUT[d * P:(d + 1) * P, qs], in_=ot)
    ffn_ps.release()
    w2p.release()
    outp.release()
    hp.release()
    x2p.release()

    dram.release()
    consts.release()


def _shard_inputs(x, Wq, bq, Wk, bk, Wv, bv, Wo, bo, g1, be1, g2, be2,
                  W1, bf1, W2, bf2):
    bf = ml_dtypes.bfloat16
    f32 = np.float32

    def colmajor(v, cols):
        return np.ascontiguousarray(v.reshape(cols, P).T).astype(f32)

    sels = np.zeros((97, 256), f32)
    sels[0, 0:64] = 1.0
    sels[32, 64:128] = 1.0
    sels[64, 128:192] = 1.0
    sels[96, 192:256] = 1.0
    kk = np.arange(P)[:, None]
    qq = np.arange(QW)[None, :]
    masks = np.stack(
        [(qq - kk - 128 * i >= 0) for i in range(4)], axis=1).astype(bf)
    W1T = np.ascontiguousarray(W1.T).astype(f32)
    W2T = np.ascontiguousarray(W2.T).astype(bf)
    shared = dict(
        BO=colmajor(bo, DT), G1=colmajor(g1, DT), BE1=colmajor(be1, DT),
        G2=colmajor(g2, DT), BE2=colmajor(be2, DT), W1T=W1T,
        BF1=colmajor(bf1, FT), W2T=W2T, BF2=colmajor(bf2, DT),
        ONESC=np.ones((P, 1), f32), ONESCB=np.ones((P, 1), bf),
        ONESR=np.ones((1, P), f32), ONESRB=np.ones((1, P), bf),
        SELS=sels, R40=np.zeros((97, QW), f32), MASKS=masks,
    )
    in_maps = []
    for c in range(8):
        b, hh = c // 2, c % 2
        heads = slice(hh * HC, (hh + 1) * HC)
        xt = np.ascontiguousarray(x[b].T).astype(f32)
        m = dict(shared)
        m["XT"] = xt
        m["XRES"] = np.ascontiguousarray(xt[:, hh * SH:(hh + 1) * SH])
        m["WQT"] = np.ascontiguousarray(
            Wq[heads].reshape(FTOK, D).T).astype(bf)
        m["WKT"] = np.ascontiguousarray(
            Wk[heads].reshape(FTOK, D).T).astype(bf)
        m["WVT"] = np.ascontiguousarray(
            Wv[heads].reshape(FTOK, D).T).astype(bf)
        m["BQ"] = colmajor(bq[heads].reshape(FTOK), NPAIR)
        m["BK"] = colmajor(bk[heads].reshape(FTOK), NPAIR)
        m["BV"] = bv[heads].reshape(1, FTOK).astype(bf)
        m["WOT"] = np.ascontiguousarray(
            Wo.T[hh * FTOK:(hh + 1) * FTOK]).astype(f32)
        in_maps.append(m)
    return in_maps


def kernel(**inputs):
    from concourse.bass_utils import run_bass_kernel_spmd

    inputs = {k: np.asarray(v) for k, v in inputs.items()}
    if "nc" not in _CACHE:
        _CACHE["nc"] = _build_program()
    nc = _CACHE["nc"]
    in_maps = _shard_inputs(**inputs)
    res = run_bass_kernel_spmd(nc, in_maps, core_ids=list(range(8)))
    _CACHE["last_result"] = res
    y = np.empty((B, S, D), dtype=np.float32)
    for c in range(8):
        b, hh = c // 2, c % 2
        y[b, hh * SH:(hh + 1) * SH, :] = res.results[c]["OUT"].T
    return y


# revision 17
# speedup vs baseline: 3.4743x; 3.4743x over previous
"""Self-contained Trainium2 Bass kernel for a dense transformer block.

Shapes (hardcoded from the problem spec):
  x: [4, 2048, 1024], 16 heads x 64, FFN 4096, f32.

Sharding over 8 NeuronCores: core c -> (batch b=c//2, head-half hh=c%2).
Each core computes LN1 on the full sequence of its batch, Q/K/V + causal
attention for its 8 heads over the full sequence, a partial output
projection, then a ReduceScatter within the (2b, 2b+1) pair splits tokens
in half for the residual + LN2 + FFN, producing 1024 output tokens per
core. Everything on-device runs in a transposed [feature, token] layout so
no transposes are ever needed; per-token LN/softmax statistics are reduced
with ones-matmuls on the tensor engine and broadcast back with K=1/K=97
matmuls.
"""

import numpy as np
import ml_dtypes

D = 1024
S = 2048
B = 4
SZ = 64
FF = 4096
P = 128
HC = 8            # heads per core
NPAIR = 4         # head pairs per core
NGRP = 2          # groups of 4 heads
DT = D // P       # 8 d-tiles
FT = FF // P      # 32 ff tiles
TBLK = S // P     # 16 token blocks
QW = 512          # q chunk width
QC = S // QW      # 4 q chunks
SH = S // 2       # tokens per core after reduce-scatter
FTOK = 512        # features (heads*SZ) per core

_CACHE = {}


def _build_program(for_sim=False, reps=1, phase_limit=99):
    import concourse.bacc as bacc
    import concourse.mybir as mybir
    import concourse.tile as tile

    dt = mybir.dt
    F32R = dt.float32r
    BF16 = dt.bfloat16
    AF = mybir.ActivationFunctionType
    ALU = mybir.AluOpType

    nc = bacc.Bacc("TRN2", debug=False)

    def din(name, shape, dtype=F32R):
        return nc.dram_tensor(name, shape, dtype, kind="ExternalInput")

    XT = din("XT", [D, S])
    XRES = din("XRES", [D, SH])
    WQT = din("WQT", [P, DT * FTOK], BF16)
    WKT = din("WKT", [P, DT * FTOK], BF16)
    WVT = din("WVT", [P, DT * FTOK], BF16)
    F32 = dt.float32
    BQ = din("BQ", [P, NPAIR], F32)
    BK = din("BK", [P, NPAIR], F32)
    BV = din("BV", [1, FTOK], BF16)
    WOT = din("WOT", [P, NPAIR * D])
    BO = din("BO", [P, DT], F32)
    G1 = din("G1", [P, DT], F32)
    BE1 = din("BE1", [P, DT], F32)
    G2 = din("G2", [P, DT], F32)
    BE2 = din("BE2", [P, DT], F32)
    W1T = din("W1T", [FT, P, DT * P])
    BF1 = din("BF1", [P, FT], F32)
    W2T = din("W2T", [DT, P, FT * P], BF16)
    BF2 = din("BF2", [P, DT], F32)
    ONESC = din("ONESC", [P, 1])
    ONESCB = din("ONESCB", [P, 1], BF16)
    ONESR = din("ONESR", [1, P])
    ONESRB = din("ONESRB", [1, P], BF16)
    SELS = din("SELS", [97, 2 * P])
    R40 = din("R40", [97, QW])
    MASKS = din("MASKS", [P, 4, QW], BF16)
    OUT = nc.dram_tensor("OUT", [D, SH], F32R, kind="ExternalOutput")

    io = locals()
    io["for_sim"] = for_sim
    io["phase_limit"] = phase_limit
    with tile.TileContext(nc) as tc:
        _body(nc, tc, mybir, io, reps=reps)
    nc.compile()
    return nc


def _body(nc, tc, mybir, io, reps=1):
    dt = mybir.dt
    F32R = dt.float32r
    F32 = dt.float32
    BF16 = dt.bfloat16
    AF = mybir.ActivationFunctionType
    ALU = mybir.AluOpType
    act = nc.scalar.activation
    mm = nc.tensor.matmul
    vec = nc.vector

    XT, XRES = io["XT"], io["XRES"]
    WQT, WKT, WVT = io["WQT"], io["WKT"], io["WVT"]
    BQ, BK, BV = io["BQ"], io["BK"], io["BV"]
    WOT, BO = io["WOT"], io["BO"]
    G1, BE1, G2, BE2 = io["G1"], io["BE1"], io["G2"], io["BE2"]
    W1T, BF1, W2T, BF2 = io["W1T"], io["BF1"], io["W2T"], io["BF2"]
    OUT = io["OUT"]

    # ---------------- constants / biases (whole kernel) ----------------
    consts = tc.alloc_tile_pool(name="consts", bufs=1)
    dram = tc.alloc_tile_pool(name="dram", bufs=1, space="DRAM")

    ones_col = consts.tile([P, 1], F32R)
    nc.sync.dma_start(out=ones_col, in_=io["ONESC"].ap())
    ones_col_bf = consts.tile([P, 1], BF16)
    nc.sync.dma_start(out=ones_col_bf, in_=io["ONESCB"].ap())
    ones_row = consts.tile([1, P], F32R)
    nc.sync.dma_start(out=ones_row, in_=io["ONESR"].ap())
    ones_row_bf = consts.tile([1, P], BF16)
    nc.sync.dma_start(out=ones_row_bf, in_=io["ONESRB"].ap())

    # selector matrices to broadcast reciprocal denominators (rows 0/32/64/96
    # of r4) to the 64-row bands of a head pair
    sels = consts.tile([97, 2 * P], F32R)
    nc.sync.dma_start(out=sels, in_=io["SELS"].ap())
    selA = sels[:, 0:P]
    selB = sels[:, P:2 * P]
    r4 = consts.tile([97, QW], F32R)
    nc.sync.dma_start(out=r4, in_=io["R40"].ap())

    # causal masks for the 4 diagonal k-tiles of a q-chunk:
    # keep exp[k, q] where (q - k - 128*i) >= 0
    maskt = consts.tile([P, 4, QW], BF16)
    nc.sync.dma_start(out=maskt, in_=io["MASKS"].ap())
    masks = [maskt[:, i, :] for i in range(4)]

    def cload(name, src, shape, dtype=F32):
        t = consts.tile(shape, dtype, tag=name)
        nc.sync.dma_start(out=t, in_=src.ap())
        return t

    bq_sb = cload("bq", BQ, [P, NPAIR])
    bk_sb = cload("bk", BK, [P, NPAIR])
    bv_sb = cload("bv", BV, [1, FTOK], BF16)
    bo_sb = cload("bo", BO, [P, DT])
    g1_sb = cload("g1", G1, [P, DT])
    be1_sb = cload("be1", BE1, [P, DT])
    g2_sb = cload("g2", G2, [P, DT])
    be2_sb = cload("be2", BE2, [P, DT])
    bf1_sb = cload("bf1", BF1, [P, FT])
    bf2_sb = cload("bf2", BF2, [P, DT])

    p_dram = dram.tile([2, D, SH], F32R)
    r_dram = dram.tile([D, SH], F32R)

    consts_env = (XT, XRES, WQT, WKT, WVT, OUT, W1T, W2T,
                  ones_col, ones_row, ones_row_bf, ones_col_bf, selA, selB,
                  r4, masks, bq_sb, bk_sb, bv_sb, bo_sb, g1_sb, be1_sb,
                  g2_sb, be2_sb, bf1_sb, bf2_sb, p_dram, r_dram, WOT)

    for _rep in range(reps):
        _compute_once(nc, tc, mybir, io, consts_env)
    dram.release()
    consts.release()


def _compute_once(nc, tc, mybir, io, env):
    dt = mybir.dt
    F32R = dt.float32r
    F32 = dt.float32
    BF16 = dt.bfloat16
    AF = mybir.ActivationFunctionType
    ALU = mybir.AluOpType
    act = nc.scalar.activation
    mm = nc.tensor.matmul
    vec = nc.vector
    PL = io.get("phase_limit", 99)
    (XT, XRES, WQT, WKT, WVT, OUT, W1T, W2T,
     ones_col, ones_row, ones_row_bf, ones_col_bf, selA, selB, r4, masks,
     bq_sb, bk_sb, bv_sb, bo_sb, g1_sb, be1_sb, g2_sb, be2_sb, bf1_sb,
     bf2_sb, p_dram, r_dram, WOT) = env

    # ---------------- phase B: LN1 + QKV ----------------
    xn1p = tc.alloc_tile_pool(name="xn1p", bufs=1)
    wqkv = tc.alloc_tile_pool(name="wqkv", bufs=1)
    xn1 = xn1p.tile([P, DT, S], BF16)
    wq_sb = wqkv.tile([P, DT, FTOK], BF16, tag="wq")
    wk_sb = wqkv.tile([P, DT, FTOK], BF16, tag="wk")
    wv_sb = wqkv.tile([P, DT, FTOK], BF16, tag="wv")
    nc.sync.dma_start(out=wq_sb, in_=WQT.ap().rearrange("p (t f) -> p t f", t=DT))
    nc.sync.dma_start(out=wk_sb, in_=WKT.ap().rearrange("p (t f) -> p t f", t=DT))
    nc.sync.dma_start(out=wv_sb, in_=WVT.ap().rearrange("p (t f) -> p t f", t=DT))

    # storage that lives QKV -> attention (right side of the heap)
    kqv = tc.alloc_tile_pool(name="kqv", bufs=1, side="right")
    kt_sb = kqv.tile([P, NPAIR, S], BF16, tag="kt")
    qt_sb = kqv.tile([P, NPAIR, S], BF16, tag="qt")
    v_sb = kqv.tile([P, TBLK, FTOK], BF16, tag="v")

    # LN1: stream x^T tiles, per-token stats via ones-matmuls, normalize
    xa = tc.alloc_tile_pool(name="xa", bufs=2 * DT, side="right")
    sqp = tc.alloc_tile_pool(name="sqp", bufs=3, side="right")
    stp = tc.alloc_tile_pool(name="stp", bufs=2, side="right")
    ln_ps = tc.alloc_tile_pool(name="ln_ps", bufs=2, space="PSUM")
    lnb_ps = tc.alloc_tile_pool(name="lnb_ps", bufs=4, space="PSUM")

    def ln_stats_and_apply(get_in_tile, out_tile, g_sb, be_sb, n_chunks):
        """LN over feature dim for transposed [d, token] tiles.

        get_in_tile(d, qc) -> [P, QW] tile of raw x^T; writes normalized
        bf16/f32r out_tile[:, d, qc*QW:...].
        """
        for qc in range(n_chunks):
            xts = [get_in_tile(d, qc) for d in range(DT)]
            ps_s = ln_ps.tile([1, QW], dt.float32, tag="stat")
            ps_q = ln_ps.tile([1, QW], dt.float32, tag="stat")
            for d in range(DT):
                mm(ps_s, ones_col, xts[d], start=(d == 0), stop=(d == DT - 1))
            for d in range(DT):
                sq = sqp.tile([P, QW], F32R, tag="sq")
                vec.tensor_mul(sq, xts[d], xts[d])
                mm(ps_q, ones_col, sq, start=(d == 0), stop=(d == DT - 1))
            mu = stp.tile([1, QW], F32R, tag="mu")
            act(out=mu, in_=ps_s, func=AF.Copy, scale=1.0 / D)
            msq = stp.tile([1, QW], F32R, tag="msq")
            act(out=msq, in_=ps_q, func=AF.Copy, scale=1.0 / D)
            mu2 = stp.tile([1, QW], F32R, tag="mu2")
            vec.tensor_mul(mu2, mu, mu)
            var = stp.tile([1, QW], F32R, tag="var")
            vec.tensor_sub(var, msq, mu2)
            sd = stp.tile([1, QW], F32R, tag="sd")
            # unbiased variance (ddof=1), eps added after sqrt
            act(out=sd, in_=var, func=AF.Sqrt, scale=float(D) / (D - 1))
            vec.tensor_scalar_add(out=sd, in0=sd, scalar1=1e-8)
            rstd = stp.tile([1, QW], F32R, tag="rstd")
            with nc.allow_low_precision(reason="f32r output is f32-width"):
                vec.reciprocal(out=rstd, in_=sd)
            ps_mu = lnb_ps.tile([P, QW], dt.float32, tag="bc")
            mm(ps_mu, ones_row, mu)
            ps_rs = lnb_ps.tile([P, QW], dt.float32, tag="bc")
            mm(ps_rs, ones_row, rstd)
            for d in range(DT):
                t1 = sqp.tile([P, QW], F32R, tag="sq")
                vec.tensor_sub(t1, xts[d], ps_mu)
                vec.tensor_mul(t1, t1, ps_rs)
                vec.tensor_scalar(
                    out=out_tile[:, d, qc * QW:(qc + 1) * QW], in0=t1,
                    scalar1=g_sb[:, d:d + 1], scalar2=be_sb[:, d:d + 1],
                    op0=ALU.mult, op1=ALU.add,
                )

    def ln1_in(d, qc):
        t = xa.tile([P, QW], F32R, tag="xa")
        nc.sync.dma_start(out=t, in_=XT[d * P:(d + 1) * P, qc * QW:(qc + 1) * QW])
        return t

    ln_stats_and_apply(ln1_in, xn1, g1_sb, be1_sb, QC if PL >= 1 else 0)
    lnb_ps.release()
    ln_ps.release()

    # QKV projections (bf16). K^T/Q^T: [2*SZ, token] pair-packed tiles.
    qkv_ps = tc.alloc_tile_pool(name="qkv_ps", bufs=3, space="PSUM")
    for p in range(NPAIR if PL >= 2 else 0):
        for qc in range(QC):
            qs = slice(qc * QW, (qc + 1) * QW)
            for dst, w, b in ((kt_sb, wk_sb, bk_sb), (qt_sb, wq_sb, bq_sb)):
                ps = qkv_ps.tile([P, QW], dt.float32, tag="qkv")
                for d in range(DT):
                    mm(ps, w[:, d, p * P:(p + 1) * P], xn1[:, d, qs],
                       start=(d == 0), stop=(d == DT - 1))
                act(out=dst[:, p, qs], in_=ps, func=AF.Identity, bias=b[:, p:p + 1])
    # V in token-major layout [token, head*sz], bias via rank-1 ones x bv
    for tb in range(TBLK if PL >= 2 else 0):
        for fc in range(2):
            fs = slice(fc * 256, (fc + 1) * 256)
            ps = qkv_ps.tile([P, 256], dt.float32, tag="qkvv")
            for d in range(DT):
                mm(ps, xn1[:, d, tb * P:(tb + 1) * P], wv_sb[:, d, fs],
                   start=(d == 0), stop=False)
            mm(ps, ones_row_bf, bv_sb[:, fs], start=False, stop=True)
            act(out=v_sb[:, tb, fs], in_=ps, func=AF.Copy)

    qkv_ps.release()
    stp.release()
    sqp.release()
    xa.release()
    wqkv.release()
    xn1p.release()

    # ---------------- attention ----------------
    # all in transposed score layout: exp_t[k, q] = exp(K q^T / 8), causal
    # masked; o^T accumulated per pair; denominators via col-tiled
    # ones-matmuls into psum_d rows {0,32,64,96}.
    ot_p = tc.alloc_tile_pool(name="ot_p", bufs=1)
    ot_sb = ot_p.tile([P, NPAIR, S], F32R)
    expp = tc.alloc_tile_pool(name="expp", bufs=3)
    sc_ps = tc.alloc_tile_pool(name="sc_ps", bufs=5, space="PSUM")
    o_ps = tc.alloc_tile_pool(name="o_ps", bufs=2, space="PSUM")
    d_ps = tc.alloc_tile_pool(name="d_ps", bufs=1, space="PSUM")

    for g in range(NGRP if PL >= 3 else 0):
        for qc in range(QC):
            qs = slice(qc * QW, (qc + 1) * QW)
            ktiles = 4 * (qc + 1)
            ps_o = [o_ps.tile([P, QW], dt.float32, tag="o", name=f"ps_o{_i}") for _i in range(2)]
            ps_d = d_ps.tile([97, QW], dt.float32, tag="d")
            for kt in range(ktiles):
                ks = slice(kt * P, (kt + 1) * P)
                exp_t = expp.tile([P, 4, QW], BF16, tag="exp")
                for lp in range(2):
                    pr = 2 * g + lp
                    for half in range(2):
                        hs = slice(half * SZ, (half + 1) * SZ)
                        ps_s = sc_ps.tile([P, QW], dt.float32, tag="sc")
                        mm(ps_s, kt_sb[hs, pr, ks], qt_sb[hs, pr, qs])
                        act(out=exp_t[:, 2 * lp + half, :], in_=ps_s,
                            func=AF.Exp, scale=0.125)
                if kt >= ktiles - 4:
                    mi = kt - (ktiles - 4)
                    for lh in range(4):
                        vec.tensor_mul(exp_t[:, lh, :], exp_t[:, lh, :], masks[mi])
                first, last = kt == 0, kt == ktiles - 1
                for lp in range(2):
                    pr = 2 * g + lp
                    for half in range(2):
                        mm(ps_o[lp][half * SZ:(half + 1) * SZ, :],
                           v_sb[:, kt, pr * P + half * SZ:pr * P + (half + 1) * SZ],
                           exp_t[:, 2 * lp + half, :],
                           start=first, stop=last, skip_group_check=True)
                for lh in range(4):
                    mm(ps_d[32 * lh:32 * lh + 1, :], ones_col_bf, exp_t[:, lh, :],
                       start=first, stop=last, tile_position=(0, 32 * lh),
                       skip_group_check=True)
            with nc.allow_low_precision(reason="f32r output is f32-width"):
                for lh in range(4):
                    vec.reciprocal(out=r4[32 * lh:32 * lh + 1, :],
                                   in_=ps_d[32 * lh:32 * lh + 1, :])
            for lp, sel in ((0, selA), (1, selB)):
                ps_b = sc_ps.tile([P, QW], dt.float32, tag="sc")
                mm(ps_b, sel, r4)
                ob = expp.tile([P, QW], F32R, tag="onorm")
                act(out=ob, in_=ps_o[lp], func=AF.Copy)
                vec.tensor_mul(ot_sb[:, 2 * g + lp, qs], ob, ps_b)

    d_ps.release()
    o_ps.release()
    sc_ps.release()
    expp.release()
    kqv.release()

    # ---------------- partial Wo -> p_dram, ReduceScatter ----------------
    wop = tc.alloc_tile_pool(name="wop", bufs=1)
    wo_sb = wop.tile([P, NPAIR, D], F32R)
    nc.sync.dma_start(out=wo_sb, in_=WOT.ap().rearrange("p (t d) -> p t d", t=NPAIR))
    pst = tc.alloc_tile_pool(name="pst", bufs=3)
    wo_ps = tc.alloc_tile_pool(name="wo_ps", bufs=2, space="PSUM")
    for d in range(DT if PL >= 4 else 0):
        for qc in range(QC):
            qs = slice(qc * QW, (qc + 1) * QW)
            ps = wo_ps.tile([P, QW], dt.float32, tag="wo")
            for f in range(NPAIR):
                mm(ps, wo_sb[:, f, d * P:(d + 1) * P], ot_sb[:, f, qs],
                   start=(f == 0), stop=(f == NPAIR - 1))
            st = pst.tile([P, QW], F32R, tag="pst")
            act(out=st, in_=ps, func=AF.Copy)
            nc.sync.dma_start(
                out=p_dram[qc // 2, d * P:(d + 1) * P,
                           (qc % 2) * QW:(qc % 2) * QW + QW],
                in_=st)
    wo_ps.release()
    pst.release()
    wop.release()
    ot_p.release()

    if PL >= 5:
        if io.get("for_sim"):
            nc.sync.dma_start(out=r_dram, in_=p_dram[0])
        else:
            nc.gpsimd.collective_compute(
                "ReduceScatter", ALU.add,
                replica_groups=[[0, 1], [2, 3], [4, 5], [6, 7]],
                ins=[p_dram.opt()], outs=[r_dram.opt()],
            )

    # ---------------- residual + LN2 ----------------
    x2p = tc.alloc_tile_pool(name="x2p", bufs=1)
    x2 = x2p.tile([P, DT, SH], F32R)
    hp = tc.alloc_tile_pool(name="hp", bufs=1)
    h_sb = hp.tile([P, FT, SH], BF16)
    xn2p = tc.alloc_tile_pool(name="xn2p", bufs=1)
    xn2 = xn2p.tile([P, DT, SH], F32R)

    rsp = tc.alloc_tile_pool(name="rsp", bufs=6, side="right")
    for d in range(DT if PL >= 5 else 0):
        for qc in range(SH // QW):
            qs = slice(qc * QW, (qc + 1) * QW)
            rt = rsp.tile([P, QW], F32R, tag="rt")
            nc.sync.dma_start(out=rt, in_=r_dram[d * P:(d + 1) * P, qs])
            xr = rsp.tile([P, QW], F32R, tag="xr")
            nc.sync.dma_start(out=xr, in_=XRES[d * P:(d + 1) * P, qs])
            vec.tensor_add(rt, rt, xr)
            act(out=x2[:, d, qs], in_=rt, func=AF.Identity, bias=bo_sb[:, d:d + 1])
    rsp.release()

    sqp2 = tc.alloc_tile_pool(name="sqp2", bufs=3, side="right")
    stp2 = tc.alloc_tile_pool(name="stp2", bufs=2, side="right")
    ln_ps2 = tc.alloc_tile_pool(name="ln_ps2", bufs=2, space="PSUM")
    lnb_ps2 = tc.alloc_tile_pool(name="lnb_ps2", bufs=4, space="PSUM")

    # reuse LN helper with x2 slices as inputs
    sqp_ref, stp_ref = sqp2, stp2

    def ln2_in(d, qc):
        return x2[:, d, qc * QW:(qc + 1) * QW]

    def ln_stats_and_apply2(get_in_tile, out_tile, g_sb, be_sb, n_chunks):
        for qc in range(n_chunks):
            xts = [get_in_tile(d, qc) for d in range(DT)]
            ps_s = ln_ps2.tile([1, QW], dt.float32, tag="stat")
            ps_q = ln_ps2.tile([1, QW], dt.float32, tag="stat")
            for d in range(DT):
                mm(ps_s, ones_col, xts[d], start=(d == 0), stop=(d == DT - 1))
            for d in range(DT):
                sq = sqp_ref.tile([P, QW], F32R, tag="sq")
                vec.tensor_mul(sq, xts[d], xts[d])
                mm(ps_q, ones_col, sq, start=(d == 0), stop=(d == DT - 1))
            mu = stp_ref.tile([1, QW], F32R, tag="mu")
            act(out=mu, in_=ps_s, func=AF.Copy, scale=1.0 / D)
            msq = stp_ref.tile([1, QW], F32R, tag="msq")
            act(out=msq, in_=ps_q, func=AF.Copy, scale=1.0 / D)
            mu2 = stp_ref.tile([1, QW], F32R, tag="mu2")
            vec.tensor_mul(mu2, mu, mu)
            var = stp_ref.tile([1, QW], F32R, tag="var")
            vec.tensor_sub(var, msq, mu2)
            sd = stp_ref.tile([1, QW], F32R, tag="sd")
            act(out=sd, in_=var, func=AF.Sqrt, scale=float(D) / (D - 1))
            vec.tensor_scalar_add(out=sd, in0=sd, scalar1=1e-8)
            rstd = stp_ref.tile([1, QW], F32R, tag="rstd")
            with nc.allow_low_precision(reason="f32r output is f32-width"):
                vec.reciprocal(out=rstd, in_=sd)
            ps_mu = lnb_ps2.tile([P, QW], dt.float32, tag="bc")
            mm(ps_mu, ones_row, mu)
            ps_rs = lnb_ps2.tile([P, QW], dt.float32, tag="bc")
            mm(ps_rs, ones_row, rstd)
            for d in range(DT):
                t1 = sqp_ref.tile([P, QW], F32R, tag="sq")
                vec.tensor_sub(t1, xts[d], ps_mu)
                vec.tensor_mul(t1, t1, ps_rs)
                vec.tensor_scalar(
                    out=out_tile[:, d, qc * QW:(qc + 1) * QW], in0=t1,
                    scalar1=g_sb[:, d:d + 1], scalar2=be_sb[:, d:d + 1],
                    op0=ALU.mult, op1=ALU.add,
                )

    ln_stats_and_apply2(ln2_in, xn2, g2_sb, be2_sb, (SH // QW) if PL >= 5 else 0)
    lnb_ps2.release()
    ln_ps2.release()

    # ---------------- FFN ----------------
    w1p = tc.alloc_tile_pool(name="w1p", bufs=3, side="right")
    ffn_ps = tc.alloc_tile_pool(name="ffn_ps", bufs=3, space="PSUM")
    w1_view = W1T.ap().rearrange("r p (t c) -> r p t c", t=DT)
    for fr in range(FT if PL >= 6 else 0):
        w1t = w1p.tile([P, DT, P], F32R, tag="w1")
        nc.sync.dma_start(out=w1t, in_=w1_view[fr])
        for qc in range(SH // QW):
            qs = slice(qc * QW, (qc + 1) * QW)
            ps = ffn_ps.tile([P, QW], dt.float32, tag="ffn")
            for d in range(DT):
                mm(ps, w1t[:, d, :], xn2[:, d, qs],
                   start=(d == 0), stop=(d == DT - 1))
            act(out=h_sb[:, fr, qs], in_=ps, func=AF.Gelu,
                bias=bf1_sb[:, fr:fr + 1])
    w1p.release()
    stp2.release()
    sqp2.release()
    xn2p.release()

    w2p = tc.alloc_tile_pool(name="w2p", bufs=2, side="right")
    outp = tc.alloc_tile_pool(name="outp", bufs=3)
    w2_view = W2T.ap().rearrange("r p (t c) -> r p t c", t=FT)
    for d in range(DT if PL >= 7 else 0):
        w2t = w2p.tile([P, FT, P], BF16, tag="w2")
        nc.sync.dma_start(out=w2t, in_=w2_view[d])
        for qc in range(SH // QW):
            qs = slice(qc * QW, (qc + 1) * QW)
            ps = ffn_ps.tile([P, QW], dt.float32, tag="ffn")
            for f in range(FT):
                mm(ps, w2t[:, f, :], h_sb[:, f, qs],
                   start=(f == 0), stop=(f == FT - 1))
            ot = outp.tile([P, QW], F32R, tag="ot")
            act(out=ot, in_=ps, func=AF.Identity, bias=bf2_sb[:, d:d + 1])
            vec.tensor_add(ot, ot, x2[:, d, qs])
            nc.sync.dma_start(out=OUT[d * P:(d + 1) * P, qs], in_=ot)
    ffn_ps.release()
    w2p.release()
    outp.release()
    hp.release()
    x2p.release()


def _shard_inputs(x, Wq, bq, Wk, bk, Wv, bv, Wo, bo, g1, be1, g2, be2,
                  W1, bf1, W2, bf2):
    bf = ml_dtypes.bfloat16
    f32 = np.float32

    def colmajor(v, cols):
        return np.ascontiguousarray(v.reshape(cols, P).T).astype(f32)

    sels = np.zeros((97, 256), f32)
    sels[0, 0:64] = 1.0
    sels[32, 64:128] = 1.0
    sels[64, 128:192] = 1.0
    sels[96, 192:256] = 1.0
    kk = np.arange(P)[:, None]
    qq = np.arange(QW)[None, :]
    masks = np.stack(
        [(qq - kk - 128 * i >= 0) for i in range(4)], axis=1).astype(bf)
    # device-tile layouts: per-partition-contiguous DMA reads
    W1T = np.ascontiguousarray(
        W1.T.reshape(DT, P, FT, P).transpose(2, 1, 0, 3).reshape(FT, P, DT * P)
    ).astype(f32)
    W2T = np.ascontiguousarray(
        W2.T.reshape(FT, P, DT, P).transpose(2, 1, 0, 3).reshape(DT, P, FT * P)
    ).astype(bf)
    shared = dict(
        BO=colmajor(bo, DT), G1=colmajor(g1, DT), BE1=colmajor(be1, DT),
        G2=colmajor(g2, DT), BE2=colmajor(be2, DT), W1T=W1T,
        BF1=colmajor(bf1, FT), W2T=W2T, BF2=colmajor(bf2, DT),
        ONESC=np.ones((P, 1), f32), ONESCB=np.ones((P, 1), bf),
        ONESR=np.ones((1, P), f32), ONESRB=np.ones((1, P), bf),
        SELS=sels, R40=np.zeros((97, QW), f32), MASKS=masks,
    )
    in_maps = []
    for c in range(8):
        b, hh = c // 2, c % 2
        heads = slice(hh * HC, (hh + 1) * HC)
        xt = np.ascontiguousarray(x[b].T).astype(f32)
        m = dict(shared)
        m["XT"] = xt
        m["XRES"] = np.ascontiguousarray(xt[:, hh * SH:(hh + 1) * SH])
        def qkvpack(W):
            wt = W[heads].reshape(FTOK, D).T          # [D, FTOK]
            return np.ascontiguousarray(
                wt.reshape(DT, P, FTOK).transpose(1, 0, 2).reshape(P, DT * FTOK)
            ).astype(bf)

        m["WQT"] = qkvpack(Wq)
        m["WKT"] = qkvpack(Wk)
        m["WVT"] = qkvpack(Wv)
        m["BQ"] = colmajor(bq[heads].reshape(FTOK), NPAIR)
        m["BK"] = colmajor(bk[heads].reshape(FTOK), NPAIR)
        m["BV"] = bv[heads].reshape(1, FTOK).astype(bf)
        wot = Wo.T[hh * FTOK:(hh + 1) * FTOK]     # [FTOK, D]
        m["WOT"] = np.ascontiguousarray(
            wot.reshape(NPAIR, P, D).transpose(1, 0, 2).reshape(P, NPAIR * D)
        ).astype(f32)
        in_maps.append(m)
    return in_maps


def kernel(**inputs):
    from concourse.bass_utils import run_bass_kernel_spmd

    inputs = {k: np.asarray(v) for k, v in inputs.items()}
    if "nc" not in _CACHE:
        _CACHE["nc"] = _build_program()
    nc = _CACHE["nc"]
    in_maps = _shard_inputs(**inputs)
    res = run_bass_kernel_spmd(nc, in_maps, core_ids=list(range(8)))
    _CACHE["last_result"] = res
    y = np.empty((B, S, D), dtype=np.float32)
    for c in range(8):
        b, hh = c // 2, c % 2
        y[b, hh * SH:(hh + 1) * SH, :] = res.results[c]["OUT"].T
    return y
